# revision 6
# baseline (speedup 1.0000x reference)
"""Trainium2 Bass kernel for the InteractPre co-attention module.

Math (reference):
    p  = relu(protein @ Wc.T + bc)           [L, 256]
    r  = relu(reactions @ W2.T + b2)         [Q, 64]
    k  = relu(p @ W1.T + b1)                 [L, 64]
    ra = r @ Wra.T + bra                     [Q, 64]
    pa = k @ Wpa.T + bpa                     [L, 64]
    A  = relu(ra[:,None,:] + pa[None,:,:]) @ Wa.T + ba   [Q, L, 64]
    r_gate = sigmoid(mean_l A);  p_gate = sigmoid(mean_q A)
    rxnfp = r*(1+r_gate); prot = max_l k*(1+p_gate)
    out = MLP(concat([rxnfp, prot]))         [Q]

Key optimization: A is never materialized.  Because the Wa matmul is linear,
    mean_l A[q] = (S_r[q]/L) @ Wa.T + ba   with  S_r[q] = sum_l relu(ra[q]+pa[l])
    mean_q A[l] = (S_p[l]/Q) @ Wa.T + ba   with  S_p[l] = sum_q relu(ra[q]+pa[l])
so only the two 64-channel sums are needed — O(Q*L*64) elementwise work, no
O(Q*L*64) HBM traffic (the naive version writes+reads ~1GB).

Sharding: L axis across the 8 cores (512 rows each).  The protein conv, the
pairwise stage and p_gate/prot are then fully local; only S_r (64x512 fp32)
needs an AllReduce, into which we piggyback the 8 per-core prot maxima.

Device layout: everything transposed (channels on partitions, tokens on the
free axis).  Pairwise stage: tile j holds tmp[c(64)x2, q(512)] =
relu(ra2 + pa2[:,j]) covering local rows l=j and l=j+256 in the two
partition halves.  Producers are split between ACT (activation w/ fused
free-axis accum -> S_p column) and DVE (tensor_scalar w/ accum); the
S_r accumulation runs on the otherwise-idle PE as a "fold" matmul
(lhsT = [I64;I64]) accumulating all 256 tiles into one PSUM bank.
"""

import os
import sys

import numpy as np

if "/opt/trn_rl_repo" not in sys.path:
    sys.path.insert(0, "/opt/trn_rl_repo")

Q = 512
L = 4096
NCORES = 8
L_LOC = L // NCORES          # 512 protein rows per core
NPAIR = L_LOC // 2           # 256 pairwise tiles per core
D = 64                       # co-attention channel count

# --- tunables -------------------------------------------------------------
# Tile j's producer goes to ACT iff (j % ACT_MOD) < ACT_NUM, else DVE.
ACT_MOD = int(os.environ.get("K_ACT_MOD", "4"))
ACT_NUM = int(os.environ.get("K_ACT_NUM", "1"))
PAIR_BF16 = os.environ.get("K_PAIR_BF16", "1") == "1"   # tmp tiles in bf16
TMP_BUFS = int(os.environ.get("K_TMP_BUFS", "8"))

_CACHE = {}


def _build():
    """Build + compile the SPMD Bass program (one program, 8 cores)."""
    import concourse.bass as bass
    import concourse.bacc as bacc
    import concourse.tile as tile
    from concourse import mybir

    f32 = mybir.dt.float32
    bf16 = mybir.dt.bfloat16
    tmp_dt = bf16 if PAIR_BF16 else f32
    AF = mybir.ActivationFunctionType
    ALU = mybir.AluOpType

    nc = bacc.Bacc("TRN2", target_bir_lowering=False, debug=False,
                   num_devices=NCORES)

    def din(name, shape, dt=f32):
        return nc.dram_tensor(name, list(shape), dt, kind="ExternalInput").ap()

    # ---- external inputs (host-prepped, transposed for lhsT use) ----
    protT = din("protT", [1024, L_LOC])        # per-core protein shard^T
    reactT = din("reactT", [256, Q])
    WcT = din("WcT", [1024, 256])
    W1T = din("W1T", [256, D])
    W2T = din("W2T", [256, D])
    WaT = din("WaT", [D, D])
    WpaT = din("WpaT", [D, D])
    WraT2 = din("WraT2", [D, 128])             # Wra.T duplicated columns
    Wf1aT = din("Wf1aT", [D, 256])             # Wf1[:, :64].T
    Wf1bT = din("Wf1bT", [D, 256])             # Wf1[:, 64:].T
    Wf2T = din("Wf2T", [256, 128])
    Wf3T = din("Wf3T", [128, 1])
    bc_d = din("bc", [256, 1])
    b1_d = din("b1", [D, 1])
    b2_d = din("b2", [D, 1])
    ba_d = din("ba", [D, 1])
    bpa_d = din("bpa", [D, 1])
    bra2_d = din("bra2", [128, 1])
    bf1_d = din("bf1", [256, 1])
    bf2_d = din("bf2", [128, 1])
    bf3_d = din("bf3", [1, 1])
    ifold_d = din("Ifold", [128, D], tmp_dt)   # [I64; I64]
    mask8_d = din("mask8", [D, NCORES])        # one-hot column = core id

    out_d = nc.dram_tensor("out", [1, Q], f32, kind="ExternalOutput").ap()

    with tile.TileContext(nc) as tc:
        with (
            tc.tile_pool(name="const", bufs=1) as cp,
            tc.tile_pool(name="work", bufs=1) as wp,
            tc.tile_pool(name="tmp", bufs=TMP_BUFS) as tp,
            tc.tile_pool(name="psum", bufs=2, space="PSUM") as ps,
            tc.tile_pool(name="psum_sr", bufs=1, space="PSUM") as ps_sr,
            tc.tile_pool(name="dram", bufs=1, space="DRAM") as dp,
        ):
            dma = nc.sync.dma_start

            # ---------------- load constants ----------------
            def cload(src, shape, dt=f32, tag=None):
                t = cp.tile(list(shape), dt, tag=tag or src.tensor.name)
                dma(t[:], src)
                return t

            WcT_sb = [cload(WcT[i * 128:(i + 1) * 128, :], [128, 256],
                            tag=f"WcT{i}") for i in range(8)]
            W1T_sb = [cload(W1T[i * 128:(i + 1) * 128, :], [128, D],
                            tag=f"W1T{i}") for i in range(2)]
            W2T_sb = [cload(W2T[i * 128:(i + 1) * 128, :], [128, D],
                            tag=f"W2T{i}") for i in range(2)]
            WaT_sb = cload(WaT, [D, D])
            WpaT_sb = cload(WpaT, [D, D])
            WraT2_sb = cload(WraT2, [D, 128])
            Wf1aT_sb = cload(Wf1aT, [D, 256])
            Wf1bT_sb = cload(Wf1bT, [D, 256])
            Wf2T_sb = [cload(Wf2T[i * 128:(i + 1) * 128, :], [128, 128],
                             tag=f"Wf2T{i}") for i in range(2)]
            Wf3T_sb = cload(Wf3T, [128, 1])
            bc_sb = [cload(bc_d[i * 128:(i + 1) * 128, :], [128, 1],
                           tag=f"bc{i}") for i in range(2)]
            b1_sb = cload(b1_d, [D, 1])
            b2_sb = cload(b2_d, [D, 1])
            ba_sb = cload(ba_d, [D, 1])
            bpa_sb = cload(bpa_d, [D, 1])
            bra2_sb = cload(bra2_d, [128, 1])
            bf1_sb = [cload(bf1_d[i * 128:(i + 1) * 128, :], [128, 1],
                            tag=f"bf1{i}") for i in range(2)]
            bf2_sb = cload(bf2_d, [128, 1])
            bf3_sb = cload(bf3_d, [1, 1])
            ifold_sb = cload(ifold_d, [128, D], tmp_dt)
            mask8_sb = cload(mask8_d, [D, NCORES])

            reactT_sb = [cload(reactT[i * 128:(i + 1) * 128, :], [128, Q],
                               tag=f"reactT{i}") for i in range(2)]
            protT_sb = [cload(protT[i * 128:(i + 1) * 128, :], [128, L_LOC],
                              tag=f"protT{i}") for i in range(8)]

            # ---------------- reaction side (replicated) ----------------
            psum_r = ps.tile([D, Q], f32, tag="ps_a")
            nc.tensor.matmul(psum_r[:], W2T_sb[0][:], reactT_sb[0][:],
                             start=True, stop=False)
            nc.tensor.matmul(psum_r[:], W2T_sb[1][:], reactT_sb[1][:],
                             start=False, stop=True)
            r_sb = wp.tile([D, Q], f32)
            nc.scalar.activation(r_sb[:], psum_r[:], AF.Relu, bias=b2_sb[:])

            # ra duplicated into both partition halves via doubled lhsT
            psum_ra2 = ps.tile([128, Q], f32, tag="ps_b")
            nc.tensor.matmul(psum_ra2[:], WraT2_sb[:], r_sb[:],
                             start=True, stop=True)
            ra2_sb = wp.tile([128, Q], tmp_dt)
            nc.scalar.activation(ra2_sb[:], psum_ra2[:], AF.Identity,
                                 bias=bra2_sb[:])

            # ---------------- protein side (sharded) ----------------
            p_sb = []
            for m in range(2):
                psum_p = ps.tile([128, L_LOC], f32, tag="ps_a")
                for i in range(8):
                    nc.tensor.matmul(
                        psum_p[:],
                        WcT_sb[i][:, m * 128:(m + 1) * 128],
                        protT_sb[i][:],
                        start=(i == 0), stop=(i == 7))
                pt = wp.tile([128, L_LOC], f32, tag=f"p{m}")
                nc.scalar.activation(pt[:], psum_p[:], AF.Relu,
                                     bias=bc_sb[m][:])
                p_sb.append(pt)

            psum_k = ps.tile([D, L_LOC], f32, tag="ps_a")
            nc.tensor.matmul(psum_k[:], W1T_sb[0][:], p_sb[0][:],
                             start=True, stop=False)
            nc.tensor.matmul(psum_k[:], W1T_sb[1][:], p_sb[1][:],
                             start=False, stop=True)
            k_sb = wp.tile([D, L_LOC], f32)
            nc.scalar.activation(k_sb[:], psum_k[:], AF.Relu, bias=b1_sb[:])

            psum_pa = ps.tile([D, L_LOC], f32, tag="ps_a")
            nc.tensor.matmul(psum_pa[:], WpaT_sb[:], k_sb[:],
                             start=True, stop=True)
            pa_sb = wp.tile([D, L_LOC], f32)
            nc.scalar.activation(pa_sb[:], psum_pa[:], AF.Identity,
                                 bias=bpa_sb[:])

            # pa2: split halves of local l onto the two partition halves
            pa2_sb = wp.tile([128, NPAIR], f32)
            dma(pa2_sb[0:D, :], pa_sb[:, 0:NPAIR])
            dma(pa2_sb[D:128, :], pa_sb[:, NPAIR:L_LOC])

            # ---------------- pairwise stage ----------------
            SpA = wp.tile([128, NPAIR], f32)    # S_p cols from ACT route
            SpD = wp.tile([128, NPAIR], f32)    # S_p cols from DVE route
            nc.gpsimd.memset(SpA[:], 0.0)
            nc.gpsimd.memset(SpD[:], 0.0)
            zeros_sb = wp.tile([128, Q], tmp_dt)
            nc.gpsimd.memset(zeros_sb[:], 0.0)

            psum_Sr = ps_sr.tile([D, Q], f32)
            for j in range(NPAIR):
                tmp = tp.tile([128, Q], tmp_dt, tag="tmp")
                col = pa2_sb[:, j:j + 1]
                if (j % ACT_MOD) < ACT_NUM:
                    nc.scalar.activation(tmp[:], ra2_sb[:], AF.Relu,
                                         bias=col,
                                         accum_out=SpA[:, j:j + 1])
                else:
                    # relu(ra2 + pa_col) = (ra2 + col) max 0, accum = free-sum
                    nc.vector.scalar_tensor_tensor(
                        tmp[:], ra2_sb[:], col, zeros_sb[:],
                        op0=ALU.add, op1=ALU.max,
                        accum_out=SpD[:, j:j + 1])
                nc.tensor.matmul(psum_Sr[:], ifold_sb[:], tmp[:],
                                 start=(j == 0), stop=(j == NPAIR - 1))

            # ---------------- p_gate / prot (local) ----------------
            Sp2 = wp.tile([128, NPAIR], f32)
            nc.vector.tensor_tensor(Sp2[:], SpA[:], SpD[:], op=ALU.add)
            Sp_hi = wp.tile([D, NPAIR], f32)
            dma(Sp_hi[:], Sp2[D:128, :])

            psum_pg = ps.tile([D, L_LOC], f32, tag="ps_a")
            nc.tensor.matmul(psum_pg[:, 0:NPAIR], WaT_sb[:], Sp2[0:D, :],
                             start=True, stop=True)
            nc.tensor.matmul(psum_pg[:, NPAIR:L_LOC], WaT_sb[:], Sp_hi[:],
                             start=True, stop=True)
            pgate_sb = wp.tile([D, L_LOC], f32)
            nc.scalar.activation(pgate_sb[:], psum_pg[:], AF.Sigmoid,
                                 bias=ba_sb[:], scale=1.0 / Q)

            g_sb = wp.tile([D, L_LOC], f32)
            nc.vector.scalar_tensor_tensor(g_sb[:], pgate_sb[:], 1.0,
                                           k_sb[:], op0=ALU.add,
                                           op1=ALU.mult)
            prot_sb = wp.tile([D, 1], f32)
            nc.vector.reduce_max(prot_sb[:], g_sb[:],
                                 axis=mybir.AxisListType.X)
            protcols_sb = wp.tile([D, NCORES], f32)
            nc.vector.tensor_scalar_mul(protcols_sb[:], mask8_sb[:],
                                        prot_sb[:])

            Sr_sb = wp.tile([D, Q], f32)
            nc.scalar.activation(Sr_sb[:], psum_Sr[:], AF.Copy)

            # ---------------- collective ----------------
            cc_in = dp.tile([D, Q + NCORES], f32)
            cc_out = dp.tile([D, Q + NCORES], f32, addr_space="Shared")
            dma(cc_in[:, 0:Q], Sr_sb[:])
            dma(cc_in[:, Q:Q + NCORES], protcols_sb[:])
            nc.gpsimd.collective_compute(
                "AllReduce", ALU.add,
                replica_groups=[list(range(NCORES))],
                ins=[cc_in[:].opt()],
                outs=[cc_out[:].opt()],
            )
            Srt_sb = wp.tile([D, Q], f32)
            dma(Srt_sb[:], cc_out[:, 0:Q])
            prota_sb = wp.tile([D, NCORES], f32)
            dma(prota_sb[:], cc_out[:, Q:Q + NCORES])

            # ---------------- r_gate / head (replicated) ----------------
            protg_sb = wp.tile([D, 1], f32)
            nc.vector.reduce_max(protg_sb[:], prota_sb[:],
                                 axis=mybir.AxisListType.X)

            psum_rg = ps.tile([D, Q], f32, tag="ps_a")
            nc.tensor.matmul(psum_rg[:], WaT_sb[:], Srt_sb[:],
                             start=True, stop=True)
            rgate_sb = wp.tile([D, Q], f32)
            nc.scalar.activation(rgate_sb[:], psum_rg[:], AF.Sigmoid,
                                 bias=ba_sb[:], scale=1.0 / L)
            rx_sb = wp.tile([D, Q], f32)
            nc.vector.scalar_tensor_tensor(rx_sb[:], rgate_sb[:], 1.0,
                                           r_sb[:], op0=ALU.add,
                                           op1=ALU.mult)

            h1_sb = []
            for m in range(2):
                psum_t = ps.tile([128, 1], f32, tag="ps_t")
                nc.tensor.matmul(psum_t[:],
                                 Wf1bT_sb[:, m * 128:(m + 1) * 128],
                                 protg_sb[:], start=True, stop=True)
                fold_sb = wp.tile([128, 1], f32, tag=f"fold{m}")
                nc.scalar.activation(fold_sb[:], psum_t[:], AF.Identity,
                                     bias=bf1_sb[m][:])
                psum_h1 = ps.tile([128, Q], f32, tag="ps_b")
                nc.tensor.matmul(psum_h1[:],
                                 Wf1aT_sb[:, m * 128:(m + 1) * 128],
                                 rx_sb[:], start=True, stop=True)
                h1l = wp.tile([128, Q], f32, tag=f"h1l{m}")
                nc.scalar.activation(h1l[:], psum_h1[:], AF.Identity,
                                     bias=fold_sb[:])
                h1 = wp.tile([128, Q], f32, tag=f"h1{m}")
                # leaky_relu(x) = max(0.01*x, x)
                nc.vector.scalar_tensor_tensor(h1[:], h1l[:], 0.01, h1l[:],
                                               op0=ALU.mult, op1=ALU.max)
                h1_sb.append(h1)

            psum_h2 = ps.tile([128, Q], f32, tag="ps_a")
            nc.tensor.matmul(psum_h2[:], Wf2T_sb[0][:], h1_sb[0][:],
                             start=True, stop=False)
            nc.tensor.matmul(psum_h2[:], Wf2T_sb[1][:], h1_sb[1][:],
                             start=False, stop=True)
            h2l_sb = wp.tile([128, Q], f32)
            nc.scalar.activation(h2l_sb[:], psum_h2[:], AF.Identity,
                                 bias=bf2_sb[:])
            h2_sb = wp.tile([128, Q], f32)
            nc.vector.scalar_tensor_tensor(h2_sb[:], h2l_sb[:], 0.01,
                                           h2l_sb[:], op0=ALU.mult,
                                           op1=ALU.max)

            psum_o = ps.tile([1, Q], f32, tag="ps_t")
            nc.tensor.matmul(psum_o[:], Wf3T_sb[:], h2_sb[:],
                             start=True, stop=True)
            out_sb = wp.tile([1, Q], f32)
            nc.scalar.activation(out_sb[:], psum_o[:], AF.Identity,
                                 bias=bf3_sb[:])
            dma(out_d, out_sb[:])

    nc.compile()
    return nc


def _get_nc():
    key = (ACT_MOD, ACT_NUM, PAIR_BF16, TMP_BUFS)
    if key not in _CACHE:
        _CACHE[key] = _build()
    return _CACHE[key]


def _prep_in_maps(inputs):
    from concourse import mybir
    bf16_np = mybir.dt.np(mybir.dt.bfloat16)
    tmp_np = bf16_np if PAIR_BF16 else np.float32

    f = lambda x: np.ascontiguousarray(np.asarray(x), dtype=np.float32)
    protein = f(inputs["protein"])[0]          # [L, 1024]
    reactions = f(inputs["reactions"])[0]      # [Q, 256]
    Wc, bc = f(inputs["Wc"]), f(inputs["bc"])
    W1, b1 = f(inputs["W1"]), f(inputs["b1"])
    W2, b2 = f(inputs["W2"]), f(inputs["b2"])
    Wa, ba = f(inputs["Wa"]), f(inputs["ba"])
    Wpa, bpa = f(inputs["Wpa"]), f(inputs["bpa"])
    Wra, bra = f(inputs["Wra"]), f(inputs["bra"])
    Wf1, bf1 = f(inputs["Wf1"]), f(inputs["bf1"])
    Wf2, bf2 = f(inputs["Wf2"]), f(inputs["bf2"])
    Wf3, bf3 = f(inputs["Wf3"]), f(inputs["bf3"])

    c = np.ascontiguousarray
    common = {
        "reactT": c(reactions.T),
        "WcT": c(Wc.T),
        "W1T": c(W1.T),
        "W2T": c(W2.T),
        "WaT": c(Wa.T),
        "WpaT": c(Wpa.T),
        "WraT2": c(np.concatenate([Wra.T, Wra.T], axis=1)),
        "Wf1aT": c(Wf1[:, :D].T),
        "Wf1bT": c(Wf1[:, D:].T),
        "Wf2T": c(Wf2.T),
        "Wf3T": c(Wf3.T),
        "bc": bc.reshape(-1, 1),
        "b1": b1.reshape(-1, 1),
        "b2": b2.reshape(-1, 1),
        "ba": ba.reshape(-1, 1),
        "bpa": bpa.reshape(-1, 1),
        "bra2": np.tile(bra.reshape(-1, 1), (2, 1)),
        "bf1": bf1.reshape(-1, 1),
        "bf2": bf2.reshape(-1, 1),
        "bf3": bf3.reshape(-1, 1),
        "Ifold": np.concatenate([np.eye(D), np.eye(D)],
                                axis=0).astype(tmp_np),
    }
    in_maps = []
    for d in range(NCORES):
        shard = c(protein[d * L_LOC:(d + 1) * L_LOC, :].T)   # [1024, L_LOC]
        mask8 = np.zeros((D, NCORES), np.float32)
        mask8[:, d] = 1.0
        in_maps.append({**common, "protT": shard, "mask8": mask8})
    return in_maps


def run(inputs, trace=False, **kw):
    from concourse import bass_utils
    nc = _get_nc()
    in_maps = _prep_in_maps(inputs)
    res = bass_utils.run_bass_kernel_spmd(
        nc, in_maps, core_ids=list(range(NCORES)), trace=trace, **kw)
    return res


def kernel(**inputs):
    res = run(inputs)
    return np.asarray(res.results[0]["out"], np.float32).reshape(-1)
